# revision 19
# baseline (speedup 1.0000x reference)
"""Trainium2 Bass kernel for nn_ConvSelfAttentionModule (B=4, C=256, H=W=64).

Reference computation per image (xf = x reshaped to [C, N], N = H*W = 4096):
    q = wq @ xf + bq                       [128, N]
    k = wk @ xf + bk                       [128, N]
    v = wv @ xf + bv                       [256, N]
    s[m, n]   = sum_d q[d, m] k[d, n]      [N, N]
    attn      = softmax_n(s)
    af[c, n]  = sum_m v[c, m] attn[m, n]   [256, N]
    out = gamma * af + x

Sharding: 8 cores = 4 images x 2 m-chunks of M=2048 attention rows.  The
kernel is ACT-bound: exp of the [2048, 4096] score block (8.4M elements at
1 elem/cycle/lane) costs ~80us/core; everything else must hide under it.

v3 highlights:
 - fp8e4 DoubleRow af for m-tiles 6-13 (pairs t=0..3): E8 = e4m3(E *
   236/rowsum[m]) is overflow-free (E <= rowsum; 236 < 240 = TRN e4m3 max),
   v8 = e4m3(5*sign(gamma)*v).  bf16 m-tiles 0-5/14-15 keep v' =
   236/rowsum * v.  Both paths yield af scaled by 5*sign(gamma)*236; host
   multiplies by |gamma|/1180 and adds x.  fp8 halves af PE time so score
   slabs stay ahead of the exp stream.
 - E tiles are [128, 2, 2048] (h-merged) so each fp8 m-tile converts with
   ONE [128, 4096] DVE tensor_scalar (2x_2p mode).
 - DMAs are descriptor-bound (~30ns per partition-row): x rides three HWDGE
   rings (sync/vector/scalar), the weight pack rides the tensor ring, and
   bk/bq are bit-packed into the weight pack (no separate tiny DMA).
   Output DMAs fan out over four rings at the tail.
 - af windows: W0 = mts 0-5 bf16 (units at points mt 6-9), WM1 = fp8 pairs
   t0,1 (points 12-13), WM2 = pairs t2,3 (points 14-15), W3 = mts 14-15
   bf16 (tail; blocks 4-7 ship via idle ACT + host add).
"""

import numpy as np
import ml_dtypes

import concourse.bass as bass  # noqa: F401  (bass types via bacc/tile)
import concourse.tile as tile
from concourse import bacc, mybir
from concourse.bass_utils import run_bass_kernel_spmd

dt = mybir.dt

P = 128          # partitions / q,k channel dim
C = 256          # channels
N = 4096         # pixels per image
M = 2048         # per-core m-chunk
MT = M // P      # 16 m-tiles
B = 4
N_CORES = 8
EXP_SHIFT = -20.0  # constant subtracted inside exp; cancels in softmax
SCALE = 236.0      # E8 = E * SCALE/rowsum  (<= 236 < 240 = TRN e4m3 max)
VSCALE = 5.0       # v8 = VSCALE*sign(gamma)*v  -> std ~4 in e4m3

FP8_MTS = set(range(6, 14))          # m-tiles 6..13 in fp8 (pairs t=0..3)
BF_SLOT = {0: 0, 1: 1, 2: 2, 3: 3, 4: 4, 5: 5, 14: 6, 15: 7}
WCOLS = 1284     # wpack cols: 1280 weights + 4 bf16-bit cols holding bk,bq

_CACHE = {}


def build_nc():
    nc = bacc.Bacc("TRN2", target_bir_lowering=False, debug=False,
                   num_devices=N_CORES)
    f32, bf16, f8 = dt.float32, dt.bfloat16, dt.float8e4
    AF = mybir.ActivationFunctionType
    DR = mybir.MatmulPerfMode.DoubleRow

    x = nc.dram_tensor("x", [C, N], bf16, kind="ExternalInput").ap()
    # Weights+biases in ONE pack (one DMA, 2.5KB-per-partition descriptors).
    # Columns: [wk0 128][wk1 128][wq0 128][wq1 128][wv0 256][wv1 256]
    #          [bv_bc 256][bk,bq fp32 bits as 4 bf16 cols]
    wpk = nc.dram_tensor("wpack", [P, WCOLS], bf16, kind="ExternalInput").ap()
    bpk = nc.dram_tensor("bpack", [P, 2], f32, kind="ExternalInput").ap()
    out = nc.dram_tensor("out_part", [C, N], bf16, kind="ExternalOutput").ap()
    out2 = nc.dram_tensor("out_part2", [P, 4, 1024], bf16,
                          kind="ExternalOutput").ap()

    with tile.TileContext(nc) as tc:
        with (
            tc.tile_pool(name="consts", bufs=1) as consts,
            tc.tile_pool(name="xs", bufs=8) as xs,
            tc.tile_pool(name="big", bufs=1) as big,
            tc.tile_pool(name="es", bufs=24) as es,
            tc.tile_pool(name="es8", bufs=8) as es8,
            tc.tile_pool(name="pss", bufs=2, space="PSUM") as pss,
            tc.tile_pool(name="psa", bufs=2, space="PSUM") as psa,
        ):
            # ---- constants ----
            # Dummy exp first so the ACT table load (~2.7us) happens before
            # anything else on the ACT queue.
            shift_t = consts.tile([P, 1], f32, name="shift_t", tag="shift_t")
            nc.vector.memset(shift_t, EXP_SHIFT)
            warm_t = consts.tile([P, 1], f32, name="warm_t", tag="warm_t")
            nc.scalar.activation(warm_t, shift_t, AF.Exp, bias=shift_t[:, 0:1],
                                 scale=1.0)

            # weight pack split: wk+wq land first so k/q matmuls start
            # early; x half-chunks 1-3 hi ride the scalar ring behind it
            wp = consts.tile([P, WCOLS], bf16, name="wp", tag="wp")
            nc.scalar.dma_start(out=wp[:, 0:512], in_=wpk[:, 0:512])
            bp = consts.tile([P, 2], f32, name="bp", tag="bp")
            nc.scalar.dma_start(out=bp, in_=bpk)
            nc.scalar.dma_start(out=wp[:, 512:WCOLS], in_=wpk[:, 512:WCOLS])
            wk_t = [wp[:, 0:128], wp[:, 128:256]]
            wq_t = [wp[:, 256:384], wp[:, 384:512]]
            wv_t = [wp[:, 512:768], wp[:, 768:1024]]
            bv_bc = wp[:, 1024:1280]
            bk_t = bp[:, 0:1]
            bq_t = bp[:, 1:2]

            rs = consts.tile([P, MT, 5], f32, name="rs", tag="rs")
            nc.vector.memset(rs, 0.0)
            rr = consts.tile([P, MT], f32, name="rr", tag="rr")

            # ---- x in 8 [128,1024] half-chunks across 3 DMA rings ----
            # (DMAs are descriptor-bound: ~3.8us of ring time each, so the
            # rings run in parallel; chunk g arrives in wave g.)
            x0q = [xs.tile([P, 512], bf16, name=f"x0q_{i}", tag="xq")
                   for i in range(4)]  # (ch-lo a, ch-hi a, ch-lo b, ch-hi b)
            nc.sync.dma_start(out=x0q[0], in_=x[0:P, 0:512])
            nc.sync.dma_start(out=x0q[1], in_=x[P:C, 0:512])
            nc.sync.dma_start(out=x0q[2], in_=x[0:P, 512:1024])
            nc.sync.dma_start(out=x0q[3], in_=x[P:C, 512:1024])
            xg = [None]
            for g in range(1, 4):
                x0 = xs.tile([P, 1024], bf16, name=f"xg0_{g}", tag="xg")
                x1 = xs.tile([P, 1024], bf16, name=f"xg1_{g}", tag="xg")
                gsl = slice(g * 1024, (g + 1) * 1024)
                nc.sync.dma_start(out=x0, in_=x[0:P, gsl])
                nc.sync.dma_start(out=x1, in_=x[P:C, gsl])
                xg.append((x0, x1))

            k_sb = big.tile([P, N], bf16, name="k_sb", tag="k_sb")
            q_sb = big.tile([P, M], bf16, name="q_sb", tag="q_sb")
            # v for bf16 m-tiles {0..5,14,15} (slots via BF_SLOT)
            v_bf = big.tile([P, 8, C], bf16, name="v_bf", tag="v_bf")
            # v for fp8 m-tiles 6..13 (slot = mt-6); DoubleRow stationary
            # slices are v_f8[:, 2t:2t+2, cc*128:(cc+1)*128]
            v_f8 = big.tile([P, 8, C], f8, name="v_f8", tag="v_f8")
            af_sb = big.tile([P, 8, 1024], bf16, name="af_sb", tag="af_sb")
            stage = big.tile([P, 4, 1024], bf16, name="stage", tag="stage")

            def k_chunk(g, half=None):
                # g==0 comes as two 512-col halves from the quarter tiles
                if g == 0:
                    xa, xb = x0q[2 * half], x0q[2 * half + 1]
                    kp = psa.tile([P, 512], f32, name=f"kp0{half}", tag="pa")
                    nc.tensor.matmul(kp, wk_t[0], xa, start=True, stop=False)
                    nc.tensor.matmul(kp, wk_t[1], xb, start=False, stop=True)
                    nc.vector.tensor_scalar_add(
                        k_sb[:, half * 512:half * 512 + 512], kp, bk_t[:, 0:1])
                    return
                x0, x1 = xg[g]
                kp = psa.tile([P, 1024], f32, name=f"kp{g}", tag="pa")
                for j in range(2):
                    sl = slice(j * 512, (j + 1) * 512)
                    nc.tensor.matmul(kp[:, sl], wk_t[0], x0[:, sl],
                                     start=True, stop=False)
                    nc.tensor.matmul(kp[:, sl], wk_t[1], x1[:, sl],
                                     start=False, stop=True)
                nc.vector.tensor_scalar_add(
                    k_sb[:, g * 1024:(g + 1) * 1024], kp, bk_t[:, 0:1])

            def q_chunk(g, half=None):
                if g == 0:
                    xa, xb = x0q[2 * half], x0q[2 * half + 1]
                    qp = psa.tile([P, 512], f32, name=f"qp0{half}", tag="pa")
                    nc.tensor.matmul(qp, wq_t[0], xa, start=True, stop=False)
                    nc.tensor.matmul(qp, wq_t[1], xb, start=False, stop=True)
                    nc.vector.tensor_scalar_add(
                        q_sb[:, half * 512:half * 512 + 512], qp, bq_t[:, 0:1])
                    return
                x0, x1 = xg[g]
                qp = psa.tile([P, 1024], f32, name=f"qp{g}", tag="pa")
                for j in range(2):
                    sl = slice(j * 512, (j + 1) * 512)
                    nc.tensor.matmul(qp[:, sl], wq_t[0], x0[:, sl],
                                     start=True, stop=False)
                    nc.tensor.matmul(qp[:, sl], wq_t[1], x1[:, sl],
                                     start=False, stop=True)
                nc.vector.tensor_scalar_add(
                    q_sb[:, g * 1024:(g + 1) * 1024], qp, bq_t[:, 0:1])

            def v_chunk(g, sub):
                # 4 m-tiles of vT (m-tiles 8g+4*sub .. +3)
                vp = psa.tile([P, 4, C], f32, name=f"vp{g}{sub}", tag="pa")
                for i in range(4):
                    t = sub * 4 + i
                    if g == 0:
                        qi = 2 * (t // 4)
                        xsl = slice((t % 4) * P, (t % 4) * P + P)
                        xlo, xhi = x0q[qi], x0q[qi + 1]
                    else:
                        xlo, xhi = xg[g]
                        xsl = slice(t * P, (t + 1) * P)
                    nc.tensor.matmul(vp[:, i], xlo[:, xsl], wv_t[0],
                                     start=True, stop=False)
                    nc.tensor.matmul(vp[:, i], xhi[:, xsl], wv_t[1],
                                     start=False, stop=True)
                for i in range(4):
                    mt = g * 8 + sub * 4 + i
                    if mt in FP8_MTS:
                        nc.vector.tensor_add(v_f8[:, mt - 6, :], vp[:, i],
                                             bv_bc)
                    else:
                        nc.vector.tensor_add(v_bf[:, BF_SLOT[mt], :],
                                             vp[:, i], bv_bc)

            e_tiles = {}    # (mt, h) -> [P, 2048] bf16
            e8_tiles = {}   # (t, h)  -> [P, 2, 2048] fp8 (pair-i major)

            def scores_slab(mt, h, s):
                # one [128,1024] slab of scores -> exp -> E (bf16), rowsum
                # partial via ACT accumulator
                if (mt, h) not in e_tiles:
                    e_tiles[(mt, h)] = es.tile([P, 2048], bf16,
                                               name=f"e{mt}_{h}", tag="e")
                e_t = e_tiles[(mt, h)]
                sp = pss.tile([P, 1024], f32, name=f"sp{mt}{h}{s}", tag="ps")
                q_l = q_sb[:, mt * P:(mt + 1) * P]
                base = h * 2048 + s * 1024
                for j in range(2):
                    nc.tensor.matmul(sp[:, j * 512:(j + 1) * 512], q_l,
                                     k_sb[:, base + j * 512:base + (j + 1) * 512],
                                     start=True, stop=True)
                nc.scalar.activation(e_t[:, s * 1024:(s + 1) * 1024], sp,
                                     AF.Exp, bias=shift_t[:, 0:1], scale=1.0,
                                     accum_out=rs[:, mt, h * 2 + s:h * 2 + s + 1])

            def scores_half(mt, j2):
                # 512-wide half of slab (mt, 0, 0): rowsum partial goes to
                # rs slot 0 (j2=0) or the spare slot 4 (j2=1)
                if (mt, 0) not in e_tiles:
                    e_tiles[(mt, 0)] = es.tile([P, 2048], bf16,
                                               name=f"e{mt}_0", tag="e")
                e_t = e_tiles[(mt, 0)]
                sp = pss.tile([P, 512], f32, name=f"sph{mt}{j2}", tag="ps")
                q_l = q_sb[:, mt * P:(mt + 1) * P]
                nc.tensor.matmul(sp, q_l,
                                 k_sb[:, j2 * 512:(j2 + 1) * 512],
                                 start=True, stop=True)
                slot = 0 if j2 == 0 else 4
                nc.scalar.activation(e_t[:, j2 * 512:(j2 + 1) * 512], sp,
                                     AF.Exp, bias=shift_t[:, 0:1], scale=1.0,
                                     accum_out=rs[:, mt, slot:slot + 1])

            def rs_chain(mt):
                # rowsum -> rr = SCALE/rowsum; then scale v (bf16 mts) or
                # convert E -> E8 fp8 in one [128,4096] op (fp8 mts)
                nc.vector.reduce_sum(rr[:, mt:mt + 1], rs[:, mt, 0:5],
                                     axis=mybir.AxisListType.X)
                nc.vector.reciprocal(rr[:, mt:mt + 1], rr[:, mt:mt + 1])
                nc.vector.tensor_scalar_mul(rr[:, mt:mt + 1],
                                            rr[:, mt:mt + 1], SCALE)
                if mt in FP8_MTS:
                    t, i = (mt - 6) // 2, (mt - 6) % 2
                    for h in range(2):
                        if (t, h) not in e8_tiles:
                            e8_tiles[(t, h)] = es8.tile(
                                [P, 2, 2048], f8, name=f"e8_{t}_{h}", tag="e8")
                        nc.vector.tensor_scalar_mul(
                            e8_tiles[(t, h)][:, i, :],
                            e_tiles[(mt, h)], rr[:, mt:mt + 1])
                else:
                    sl = BF_SLOT[mt]
                    nc.vector.tensor_scalar_mul(v_bf[:, sl, :],
                                                v_bf[:, sl, :],
                                                rr[:, mt:mt + 1])

            # ---- af units ----
            def af_mm_bf(ap, b, mts, start, stop):
                h, cc, nq = b >> 2, (b >> 1) & 1, b & 1
                for idx, mt in enumerate(mts):
                    lhs = v_bf[:, BF_SLOT[mt], cc * P:(cc + 1) * P]
                    e_t = e_tiles[(mt, h)]
                    for j in range(2):
                        nc.tensor.matmul(
                            ap[:, j * 512:(j + 1) * 512], lhs,
                            e_t[:, nq * 1024 + j * 512:nq * 1024 + (j + 1) * 512],
                            start=(start and idx == 0),
                            stop=(stop and idx == len(mts) - 1))

            def af_mm_f8(ap, b, ts, start, stop):
                h, cc, nq = b >> 2, (b >> 1) & 1, b & 1
                for idx, t in enumerate(ts):
                    lhsT = v_f8[:, 2 * t:2 * t + 2, cc * P:(cc + 1) * P]
                    e8 = e8_tiles[(t, h)]
                    base = nq * 1024
                    for j in range(2):
                        nc.tensor.matmul(
                            ap[:, j * 512:(j + 1) * 512], lhsT,
                            e8[:, :, base + j * 512:base + (j + 1) * 512],
                            start=(start and idx == 0),
                            stop=(stop and idx == len(ts) - 1),
                            perf_mode=DR)

            SHIP_ENG = [nc.sync, nc.gpsimd, nc.sync, nc.gpsimd]
            FIN_ENG = [nc.scalar, nc.sync, nc.scalar, nc.gpsimd]
            OUT2_ENG = [nc.scalar, nc.gpsimd, nc.scalar, nc.sync]

            def af_unit(win, b):
                h, cc, nq = b >> 2, (b >> 1) & 1, b & 1
                ap = psa.tile([P, 1024], f32, name=f"af{win}_{b}", tag="pa")
                dst = af_sb[:, b, :]
                osl = out[cc * P:(cc + 1) * P,
                          h * 2048 + nq * 1024:h * 2048 + (nq + 1) * 1024]
                if win == "W0":
                    af_mm_bf(ap, b, [0, 1, 2, 3, 4, 5], True, True)
                    nc.vector.tensor_copy(dst, ap)
                elif win == "WM1":
                    af_mm_f8(ap, b, [0, 1], True, True)
                    nc.vector.tensor_add(dst, ap, dst)
                elif win == "WM2":
                    af_mm_f8(ap, b, [2, 3], True, True)
                    nc.vector.tensor_add(dst, ap, dst)
                    if b >= 4:
                        # blocks 4-7 finish via ACT+host: ship the sum
                        # through WM2 now; W3's partial goes via out2
                        SHIP_ENG[b - 4].dma_start(out=osl, in_=dst)
                else:  # W3, tail
                    af_mm_bf(ap, b, [14, 15], True, True)
                    if b < 4:
                        nc.vector.tensor_add(dst, ap, dst)
                        FIN_ENG[b].dma_start(out=osl, in_=dst)
                    else:
                        # final window partial via idle ACT; host adds it
                        st = stage[:, b - 4, :]
                        nc.scalar.copy(st, ap)
                        OUT2_ENG[b - 4].dma_start(out=out2[:, b - 4, :],
                                                  in_=st)

            unit_sched = {
                (6, 0): [("W0", 0)], (6, 1): [("W0", 1)],
                (7, 0): [("W0", 2)], (7, 1): [("W0", 3)],
                (8, 0): [("W0", 4)], (8, 1): [("W0", 5)],
                (11, 0): [("W0", 6)], (11, 1): [("W0", 7)],
                (10, 0): [("WM1", 0)], (10, 1): [("WM1", 1)],
                (11, 0): [("WM1", 2)], (11, 1): [("WM1", 3)],
                (12, 0): [("WM1", 4)], (12, 1): [("WM1", 5)],
                (13, 0): [("WM1", 6)], (13, 1): [("WM1", 7)],
                (14, 0): [("WM2", 0), ("WM2", 1)],
                (14, 1): [("WM2", 2), ("WM2", 3)],
                (15, 0): [("WM2", 4), ("WM2", 5)],
                (15, 1): [("WM2", 6), ("WM2", 7)],
            }

            # ---- emission ----
            # prologue: x chunk-0 quarters let the first 512-wide score
            # halves run ~4.5us earlier; mt2/mt3's first slabs fill the ACT
            # stream until chunk 1 lands.
            k_chunk(0, 0); q_chunk(0, 0)
            scores_half(0, 0)
            scores_half(1, 0)
            k_chunk(0, 1); q_chunk(0, 1)
            scores_half(0, 1)
            scores_half(1, 1)
            scores_slab(2, 0, 0)
            scores_slab(3, 0, 0)
            k_chunk(1); q_chunk(1)
            scores_slab(0, 0, 1)
            scores_slab(1, 0, 1)
            k_chunk(2)
            scores_slab(0, 1, 0)
            scores_slab(1, 1, 0)
            k_chunk(3)
            scores_slab(0, 1, 1)
            v_chunk(0, 0)
            rs_chain(0)
            scores_slab(1, 1, 1)
            v_chunk(0, 1)
            rs_chain(1)

            done_early = {(2, 0, 0), (3, 0, 0)}

            # steady-state sweep; af unit bursts dispatched per unit_sched
            for mt in range(2, MT):
                for h in range(2):
                    for s in range(2):
                        if (mt, h, s) not in done_early:
                            scores_slab(mt, h, s)
                    for win, b in unit_sched.get((mt, h), ()):
                        af_unit(win, b)
                if mt == 4:
                    v_chunk(1, 0)
                elif mt == 5:
                    v_chunk(1, 1)
                rs_chain(mt)

            # tail: last window (m-tiles 14-15, bf16); ACT-path blocks
            # first so the idle ACT starts its copies sooner
            for b in (4, 5, 6, 7, 0, 1, 2, 3):
                af_unit("W3", b)

    nc.compile()
    return nc


def _get_nc():
    if "nc" not in _CACHE:
        _CACHE["nc"] = build_nc()
    return _CACHE["nc"]


def build_in_maps(x, wq, bq, wk, bk, wv, bv, gamma):
    bf = ml_dtypes.bfloat16
    x = np.asarray(x, np.float32)
    g = float(np.asarray(gamma).reshape(-1)[0])
    sg = VSCALE * (1.0 if g >= 0 else -1.0)
    wqT = np.asarray(wq, np.float32).T
    wkT = np.asarray(wk, np.float32).T
    wvT = (sg * np.asarray(wv, np.float32)).T
    bk2 = np.asarray(bk, np.float32).reshape(P, 1)
    bq2 = np.asarray(bq, np.float32).reshape(P, 1)
    bv_bc = np.broadcast_to((sg * np.asarray(bv, np.float32)).reshape(1, C),
                            (P, C))
    bbits = np.ascontiguousarray(
        np.concatenate([bk2, bq2], axis=1)).view(np.uint16).reshape(P, 4)
    wpack = np.concatenate(
        [wkT[0:P, :].astype(bf), wkT[P:C, :].astype(bf),
         wqT[0:P, :].astype(bf), wqT[P:C, :].astype(bf),
         wvT[0:P, :].astype(bf), wvT[P:C, :].astype(bf),
         bv_bc.astype(bf), bbits.view(bf)], axis=1)
    wpack = np.ascontiguousarray(wpack)
    bpack = np.ascontiguousarray(np.concatenate([bk2, bq2], axis=1))
    xf = x.reshape(B, C, N).astype(bf)
    in_maps = []
    for core in range(N_CORES):
        b, half = core // 2, core % 2
        xc = xf[b] if half == 0 else np.roll(xf[b], -M, axis=1)
        in_maps.append(dict(x=np.ascontiguousarray(xc), wpack=wpack,
                            bpack=bpack))
    return in_maps


def assemble(results, x, gamma):
    x = np.asarray(x, np.float32)
    g = float(np.asarray(gamma).reshape(-1)[0])
    host_scale = abs(g) / (VSCALE * SCALE)
    af = np.zeros((B, C, N), np.float32)
    for core in range(N_CORES):
        b, half = core // 2, core % 2
        part = np.asarray(results[core]["out_part"]).astype(np.float32)
        part2 = np.asarray(results[core]["out_part2"]).astype(np.float32)
        # blocks 4-7 (h=1) got their last af window shipped separately
        for j in range(4):
            cc, nq = (j >> 1) & 1, j & 1
            part[cc * P:(cc + 1) * P,
                 2048 + nq * 1024:2048 + (nq + 1) * 1024] += part2[:, j, :]
        af[b] += part if half == 0 else np.roll(part, M, axis=1)
    return (host_scale * af.reshape(x.shape) + x).astype(np.float32)


def kernel(x, wq, bq, wk, bk, wv, bv, gamma):
    nc = _get_nc()
    in_maps = build_in_maps(x, wq, bq, wk, bk, wv, bv, gamma)
    res = run_bass_kernel_spmd(nc, in_maps, core_ids=list(range(N_CORES)))
    return assemble(res.results, x, gamma)
